# revision 4
# baseline (speedup 1.0000x reference)
"""Trainium2 Bass kernel for nn_MultiHeadAttention_9912784519532 (v2).

MHA with relative position bias: b=2, n=2048, dim=512, heads=8, d_head=64,
rel table (2*512+1, 64).

Sharding: 16 (batch, head) pairs over 8 cores -> 2 heads of one batch per
core. Each core computes a partial output y_part = attn_out @ Wo_slice for
its 2 heads; host sums 4 partials per batch and adds bo.

v2 design (vs the fp32-PE-transpose baseline):
  - Both heads packed on partition halves; all 64-contraction matmuls
    (S^T, wER, per-head ops) emitted as adjacent row-tiled pairs
    (tile_position (0,0)/(64,0)) so they run concurrently in the PE.
  - The rel-pos band is read back from the DRAM wER table with the XBAR
    DMA transpose (bf16) directly in [keys, queries] orientation and
    added into the score PSUM with identity-stationary bf16 matmuls --
    no fp32 PE transposes at all.
  - V is produced transposed by the PE (full 128-contraction) and turned
    into its [keys, d] layout by 32 small XBAR transpose DMAs.
  - Softmax normalization: denominator row -> reciprocal -> broadcast via
    a rank-1 ones matmul -> fused multiply on the DVE.
  - Output projection runs at full 128 contraction (both heads at once).
"""

import numpy as np

HEADS = 8
D = 64
N = 2048
DIM = 512
WER = 1280  # width of padded/reversed rel projection table
P = 128
NT = N // P
QW = 512
SCALE = float(D) ** -0.5

_cached = {}


def _build_program():
    import concourse.bass as bass
    import concourse.mybir as mybir
    import concourse.tile as tile
    from concourse import bacc

    f32 = mybir.dt.float32
    bf16 = mybir.dt.bfloat16
    AP = bass.AP
    Exp = mybir.ActivationFunctionType.Exp

    nc = bacc.Bacc(
        "TRN2",
        target_bir_lowering=False,
        debug=False,
        enable_asserts=False,
        num_devices=8,
    )

    xT_d = nc.dram_tensor("xT", [DIM, N], bf16, kind="ExternalInput")
    wq_d = nc.dram_tensor("wq2", [DIM, P], bf16, kind="ExternalInput")
    wk_d = nc.dram_tensor("wk2", [DIM, P], bf16, kind="ExternalInput")
    wv_d = nc.dram_tensor("wv2", [DIM, P], bf16, kind="ExternalInput")
    wo_d = nc.dram_tensor("wo2", [P, DIM], bf16, kind="ExternalInput")
    relx_d = nc.dram_tensor("relx2", [P, WER], bf16, kind="ExternalInput")
    edge_d = nc.dram_tensor("edge2", [P, 2], f32, kind="ExternalInput")
    identb_d = nc.dram_tensor("identb", [P, P], bf16, kind="ExternalInput")
    identf_d = nc.dram_tensor("identf", [P, P], f32, kind="ExternalInput")
    y_d = nc.dram_tensor("y", [N, DIM], f32, kind="ExternalOutput")

    wer_d = [
        nc.dram_tensor(f"wer{h}", [N, WER], bf16, kind="Internal") for h in range(2)
    ]

    def window_range(kt):
        kb = kt * P
        return max(0, kb - 512), min(N, kb + 640)

    with tile.TileContext(nc) as tc:
        import contextlib

        ctx = contextlib.ExitStack()
        with ctx:
            const = ctx.enter_context(tc.tile_pool(name="const", bufs=1))
            big = ctx.enter_context(tc.tile_pool(name="big", bufs=1))
            cpool = ctx.enter_context(tc.tile_pool(name="copies", bufs=4))
            zpool = ctx.enter_context(tc.tile_pool(name="zps", bufs=2, space="PSUM"))
            opool = ctx.enter_context(tc.tile_pool(name="ops", bufs=2, space="PSUM"))
            apool = ctx.enter_context(tc.tile_pool(name="attn", bufs=3))
            wpool = ctx.enter_context(tc.tile_pool(name="win", bufs=1))
            spool = ctx.enter_context(tc.tile_pool(name="small", bufs=1))

            def ztile(h):
                return zpool.tile([P, QW], f32, name=f"zt{h}", tag=f"zt{h}")

            # ---- load constants / inputs (weights first: the first
            # projection matmul only needs wq + one xT chunk) ----
            wq_sb = const.tile([P, 4, P], bf16)
            nc.scalar.dma_start(wq_sb[:], wq_d.ap().rearrange("(c p) m -> p c m", p=P))
            wk_sb = const.tile([P, 4, P], bf16)
            nc.scalar.dma_start(wk_sb[:], wk_d.ap().rearrange("(c p) m -> p c m", p=P))
            wv_sb = const.tile([P, 4, P], bf16)
            nc.scalar.dma_start(wv_sb[:], wv_d.ap().rearrange("(c p) m -> p c m", p=P))
            relx_sb = const.tile([P, WER], bf16)
            nc.scalar.dma_start(relx_sb[:], relx_d.ap())
            edge_sb = const.tile([P, 2], f32)
            nc.scalar.dma_start(edge_sb[:], edge_d.ap())
            identb_sb = const.tile([P, P], bf16)
            nc.scalar.dma_start(identb_sb[:], identb_d.ap())
            identf_sb = const.tile([P, P], f32)
            nc.scalar.dma_start(identf_sb[:], identf_d.ap())
            wo_sb = const.tile([P, DIM], bf16)
            nc.scalar.dma_start(wo_sb[:], wo_d.ap())
            xt_sb = big.tile([P, 4, N], bf16)
            for half in range(2):
                for cc in range(4):
                    nc.sync.dma_start(
                        xt_sb[:, cc, half * 1024 : (half + 1) * 1024],
                        xT_d.ap()[cc * P : (cc + 1) * P, half * 1024 : (half + 1) * 1024],
                    )
            ones_sb = const.tile([1, D], f32)
            nc.vector.memset(ones_sb[:], 1.0)

            # HAM warm-up: ~3.5us of junk matmuls while the xT load streams,
            # so the projection phase starts at the full 2.4 GHz clock
            heat = zpool.tile([P, QW], f32, name="heat", tag="zt0")
            for _ in range(36):
                nc.tensor.matmul(
                    heat[:, :P], identb_sb[:], identb_sb[:], start=True, stop=True
                )

            # ---- q projection first: wER depends only on q ----
            qt2 = big.tile([P, N], bf16)
            kt2 = big.tile([P, N], bf16)
            for nch in range(4):
                tgt = ztile(0)[:]
                for cc in range(4):
                    nc.tensor.matmul(
                        tgt,
                        wq_sb[:, cc, :],
                        xt_sb[:, cc, nch * 512 : (nch + 1) * 512],
                        start=(cc == 0),
                        stop=(cc == 3),
                    )
                nc.vector.tensor_copy(qt2[:, nch * 512 : (nch + 1) * 512], tgt)

            # ---- wER tables -> DRAM, interleaved with the band transposes
            # on the same HWDGE ring so each transpose sits right behind the
            # last write it depends on (transpose kt needs rows < (kt+5)*128)
            ktp = big.tile([P, N], bf16)
            ktf = big.tile([P, N], bf16)
            v2 = big.tile([P, 2, NT, 65], bf16)
            nc.vector.memset(v2[:], 1.0)
            wins = {}
            wins2 = {}

            def issue_band(J):
                # Hybrid band staging:
                #  - query blocks J<8 (first half): query-major f32 reads
                #    (SWDGE cast) feeding fp32 PE transposes
                #  - J>=8: the same wER rows feed XBAR-transposed bf16 tiles
                #    (per key-tile) consumed by identity-add matmuls, taking
                #    load off the PE during the second half
                qb = J * P
                r0 = max(0, qb - 512)
                r1 = min(N, qb + 640)
                for h in range(2):
                    wb = wpool.tile(
                        [P, 1152], bf16, name=f"wb{h}_{J}", tag=f"wb{h}_{J % 8}"
                    )
                    src = AP(
                        tensor=wer_d[h],
                        offset=qb * (WER - 1) + 640 + r0,
                        ap=[[WER - 1, P], [1, r1 - r0]],
                    )
                    nc.sync.dma_start(wb[:, : r1 - r0], src)
                    wt = wpool.tile(
                        [P, 1152], f32, name=f"win{h}_{J}", tag=f"win{h}_{J % 8}"
                    )
                    nc.gpsimd.tensor_copy(wt[:, : r1 - r0], wb[:, : r1 - r0])
                    wins[(h, J)] = (wt, r0, r1)


            for qt in range(NT):
                wt0 = cpool.tile([P, WER], bf16, name="wer_sb0", tag="wer_sb0")
                wt1 = cpool.tile([P, WER], bf16, name="wer_sb1", tag="wer_sb1")
                for ci, (c0, cw) in enumerate(((0, 512), (512, 512), (1024, 256))):
                    p0 = ztile(0)
                    p1 = ztile(1)
                    for h, ptw in ((0, p0), (1, p1)):
                        nc.tensor.matmul(
                            ptw[:, :cw],
                            qt2[h * 64 : (h + 1) * 64, qt * P : (qt + 1) * P],
                            relx_sb[h * 64 : (h + 1) * 64, c0 : c0 + cw],
                            start=True,
                            stop=True,
                        )
                    nc.vector.tensor_copy(wt0[:, c0 : c0 + cw], p0[:, :cw])
                    nc.scalar.copy(wt1[:, c0 : c0 + cw], p1[:, :cw])
                nc.sync.dma_start(wer_d[0].ap()[qt * P : (qt + 1) * P, :], wt0[:])
                nc.sync.dma_start(wer_d[1].ap()[qt * P : (qt + 1) * P, :], wt1[:])
                # fill the copy-bound wER phase with the k/v projections,
                # using the (idle until flash) opool PSUM slots
                if qt < 4:
                    ktg = opool.tile([P, 512], f32, name="kproj", tag="o1")[:]
                    for cc in range(4):
                        nc.tensor.matmul(
                            ktg,
                            wk_sb[:, cc, :],
                            xt_sb[:, cc, qt * 512 : (qt + 1) * 512],
                            start=(cc == 0),
                            stop=(cc == 3),
                        )
                    nc.scalar.copy(kt2[:, qt * 512 : (qt + 1) * 512], ktg)
                    if qt == 3:
                        nc.vector.tensor_scalar_add(ktp[:], kt2[:], edge_sb[:, 0:1])
                        nc.vector.tensor_scalar_add(ktf[:], kt2[:], edge_sb[:, 1:2])
                vtg = opool.tile([P, 512], f32, name="vproj", tag="o0")[:, :P]
                for cc in range(4):
                    nc.tensor.matmul(
                        vtg,
                        xt_sb[:, cc, qt * P : (qt + 1) * P],
                        wv_sb[:, cc, :],
                        start=(cc == 0),
                        stop=(cc == 3),
                    )
                for h in range(2):
                    nc.vector.tensor_copy(
                        v2[:, h, qt, 0:64], vtg[:, h * 64 : h * 64 + 64]
                    )
                if qt >= 4:
                    issue_band(qt - 4)
            for kt in range(NT - 4, NT):
                issue_band(kt)

            # ---- flash attention ----
            otn = big.tile([P, N], bf16)  # normalized outT, heads packed
            NQC = N // QW
            for qc in range(NQC):
                Q0 = qc * QW
                opt = {
                    h: opool.tile([P, 512], f32, name=f"o{h}", tag=f"o{h}")
                    for h in range(2)
                }
                oths = {h: opt[h][0:65, :] for h in range(2)}
                avq = []  # software-pipelined AV matmuls, one kt behind
                for kt in range(NT):
                    kb = kt * P
                    cls = []
                    for j in range(QW // P):
                        dlt = Q0 + j * P - kb
                        cls.append("p" if dlt >= 640 else ("f" if dlt <= -640 else "w"))
                    zt = [ztile(0), ztile(1)]
                    runs = []
                    for j in range(QW // P):
                        if runs and runs[-1][2] == cls[j]:
                            runs[-1][1] += P
                        else:
                            runs.append([j * P, P, cls[j]])
                    wjs = [j for j in range(QW // P) if cls[j] == "w"]
                    pieces = []
                    nops = len(runs) + len(wjs) + len(pieces)
                    for i, (rs, wd, c) in enumerate(runs):
                        kv = {"p": ktp, "f": ktf, "w": kt2}[c]
                        for h in range(2):
                            hs = slice(h * 64, h * 64 + 64)
                            nc.tensor.matmul(
                                zt[h][:, rs : rs + wd],
                                kv[hs, kb : kb + P],
                                qt2[hs, Q0 + rs : Q0 + rs + wd],
                                start=(i == 0),
                                stop=(i == nops - 1),
                                skip_group_check=True,
                            )
                    for i, j in enumerate(wjs):
                        J = Q0 // P + j
                        for h in range(2):
                            wtf, r0, _ = wins[(h, J)]
                            nc.tensor.matmul(
                                zt[h][:, j * P : (j + 1) * P],
                                wtf[:, kb - r0 : kb - r0 + P],
                                identf_sb[:],
                                is_transpose=True,
                                start=False,
                                stop=(len(runs) + i == nops - 1),
                                skip_group_check=True,
                            )
                    for i, (rs, wd, wof) in enumerate(pieces):
                        for h in range(2):
                            nc.tensor.matmul(
                                zt[h][:, rs : rs + wd],
                                identb_sb[:],
                                wins2[(h, kt)][0][:, wof : wof + wd],
                                start=False,
                                stop=(len(runs) + len(wjs) + i == nops - 1),
                                skip_group_check=True,
                            )
                    # drain the previous kt's AV matmuls (exp already done)
                    for mmargs in avq:
                        nc.tensor.matmul(*mmargs[:3], start=mmargs[3], stop=mmargs[4])
                    avq = []
                    for h in range(2):
                        at = apool.tile([P, QW], bf16, name=f"at{h}", tag=f"at{h}")
                        nc.scalar.activation(at[:], zt[h][:], Exp, scale=SCALE)
                        avq.append(
                            (oths[h], v2[:, h, kt, :], at[:],
                             kt == 0, kt == NT - 1)
                        )
                for mmargs in avq:
                    nc.tensor.matmul(*mmargs[:3], start=mmargs[3], stop=mmargs[4])
                # normalization: den row -> rank-1 broadcast -> approx recip -> mul
                for h in range(2):
                    oth = oths[h]
                    den = spool.tile([1, 512], f32, name=f"den{h}", tag=f"den{h}")
                    nc.vector.tensor_copy(den[:], oth[64:65, :])
                    dn = opool.tile([P, 512], f32, name=f"dn{h}", tag=f"o{h}")
                    denb = dn[0:64, :]
                    nc.tensor.matmul(denb, ones_sb[:], den[:], start=True, stop=True)
                    rdb = spool.tile([64, 512], f32, name=f"rdb{h}", tag=f"rdb{h}")
                    nc.vector.reciprocal_approx_fast(rdb[:], denb)
                    nc.vector.tensor_mul(
                        otn[h * 64 : (h + 1) * 64, Q0 : Q0 + QW],
                        oth[0:64, :],
                        rdb[:],
                    )
            # ---- output projection (full 128 contraction, both heads) ----
            for nt in range(NT):
                yp = ztile(nt % 2)
                nc.tensor.matmul(
                    yp[:],
                    otn[:, nt * P : (nt + 1) * P],
                    wo_sb[:],
                    start=True,
                    stop=True,
                )
                ysb = cpool.tile([P, 512], f32, name="ysb", tag="ysb")
                nc.scalar.copy(ysb[:], yp[:])
                nc.scalar.dma_start(y_d.ap()[nt * P : (nt + 1) * P, :], ysb[:])

    nc.compile()
    return nc


def _host_prep(x, Wq, Wkv, Wo, rel_emb):
    """Build the 8 per-core input maps."""
    import ml_dtypes

    bf = ml_dtypes.bfloat16
    identb = np.eye(P, dtype=bf)
    identf = np.eye(P, dtype=np.float32)
    relX = rel_emb[np.clip(1152 - np.arange(WER), 0, 1024)].T
    relx2 = np.ascontiguousarray(np.concatenate([relX, relX], axis=0).astype(bf))
    edge = np.stack([rel_emb[1024], rel_emb[0]], axis=1)
    edge2 = np.ascontiguousarray(
        np.concatenate([edge, edge], axis=0).astype(np.float32)
    )
    Wkv_r = Wkv.reshape(DIM, 2, HEADS, D)
    in_maps = []
    for core in range(8):
        b = core // 4
        h0 = 2 * (core % 4)
        in_maps.append(
            {
                "xT": np.ascontiguousarray(x[b].T.astype(bf)),
                "wq2": np.ascontiguousarray(Wq[:, h0 * D : (h0 + 2) * D].astype(bf)),
                "wk2": np.ascontiguousarray(
                    Wkv_r[:, 0, h0 : h0 + 2].reshape(DIM, 2 * D).astype(bf)
                ),
                "wv2": np.ascontiguousarray(
                    Wkv_r[:, 1, h0 : h0 + 2].reshape(DIM, 2 * D).astype(bf)
                ),
                "wo2": np.ascontiguousarray(Wo[h0 * D : (h0 + 2) * D, :].astype(bf)),
                "relx2": relx2,
                "edge2": edge2,
                "identb": identb,
                "identf": identf,
            }
        )
    return in_maps


def kernel(x, Wq, Wkv, Wo, bo, rel_emb, _want_trace=False):
    from concourse.bass_utils import run_bass_kernel_spmd

    x = np.asarray(x)
    if "nc" not in _cached:
        _cached["nc"] = _build_program()
    nc = _cached["nc"]
    in_maps = _host_prep(
        x, np.asarray(Wq), np.asarray(Wkv), np.asarray(Wo), np.asarray(rel_emb)
    )
    res = run_bass_kernel_spmd(
        nc, in_maps, core_ids=list(range(8)), trace=_want_trace
    )
    _cached["last_result"] = res
    y = np.zeros((2, N, DIM), np.float32)
    for core in range(8):
        y[core // 4] += res.results[core]["y"]
    y += np.asarray(bo).astype(np.float32)[None, None, :]
    return y
